# revision 7
# baseline (speedup 1.0000x reference)
"""Causal MHA + out-projection Trainium2 kernel (B=64, S=512, E=256, H=4).

Sharding: data-parallel over batch, 8 batches per NeuronCore x 8 cores.

Per (batch, head):
  - S^T[k, q] = sum_d K[k,d] Q[q,d]  (chunks of 128 keys; q free dim,
    causally restricted to q >= 128*j for key-chunk j)
  - P^T = exp(S^T/8) on the scalar engine (one op per head); the causal
    triangle inside each 128x128 diagonal block is zeroed afterwards with
    one gpsimd affine_select per chunk (cheaper than mask matmuls on PE).
  - PV in q-partition layout: per q-tile i, key-chunks j<=i accumulate
    matmul(lhsT = q-tile slice of P^T, rhs = [V_h | one ones-column]) into
    O[128q, 4, 65] -- a single PSUM bank, with the softmax denominator
    landing as one per-partition column per q-tile.
  - Normalize is therefore tiny on the vector engine: a 4-column
    reciprocal_approx_fast reading the denominator column straight from
    PSUM, and ONE broadcast multiply (stride-0 free dim) writing this
    head's 64-column half of a PAIR tile X[q, 4 tiles, 128] in bf16.
  - Four [128,128] PE transpose-mode matmuls per head PAIR (not per head:
    a transpose costs its 128 output columns regardless of partition
    count, so stacking both heads' d-halves on the partition dim halves
    the transpose count) restore X^T[d-pair, q] into bf16 PSUM staging
    tag-shared with the projection output, evacuated once per pair.
  - Y[q,:] = X^T.T @ W^T in one reused 1-bank PSUM slot, egressed as bf16 on
    the vector engine; the bias row is added on the host after the gather
    (a per-feature constant -- exact up to the bf16 output rounding).

The emission is software-pipelined 3 stages deep (QK heads 2 units ahead of
the exp/PV tails, normalize one further unit behind) so every op's
dependencies are settled before it reaches the front of its strict-FIFO
engine queue. Host-side prep: qry/key are pre-transposed to [B, E, S] and
w_out to W^T so that every device DMA is contiguous.
"""

import sys

if "/opt/trn_rl_repo" not in sys.path:
    sys.path.insert(0, "/opt/trn_rl_repo")

import numpy as np

import concourse.bass as bass
import concourse.mybir as mybir
import concourse.tile as tile
from concourse import bacc
from concourse.bass_utils import run_bass_kernel_spmd

B, S, E, H = 64, 512, 256, 4
D = E // H  # 64
P = 128
NCORES = 8
BPC = B // NCORES  # 8

FP = mybir.dt.float32
BF = mybir.dt.bfloat16
TD = BF

# S^T chunk -> column offset inside the 3-bank (1280 col) psum tile.
# chunk j covers keys [128j, 128j+128), q in [128j, 512) => width 512-128j.
CHUNK_COL = [0, 512, 1024, 896]
ST_COLS = 1280  # exp() span


def attention_kernel(ctx, tc, out, qT, kT, v, wT, bo, repeat=1):
    nc = tc.nc
    AF = mybir.ActivationFunctionType
    OP = mybir.AluOpType

    consts = ctx.enter_context(tc.tile_pool(name="consts", bufs=1))
    qk_pool = ctx.enter_context(tc.tile_pool(name="qk", bufs=3))
    v_pool = ctx.enter_context(tc.tile_pool(name="v", bufs=3))
    pt_pool = ctx.enter_context(tc.tile_pool(name="pt", bufs=3))
    xt_pool = ctx.enter_context(tc.tile_pool(name="xt", bufs=2))
    yo_pool = ctx.enter_context(tc.tile_pool(name="yo", bufs=2))
    r_pool = ctx.enter_context(tc.tile_pool(name="rsb", bufs=3))
    st_psum = ctx.enter_context(tc.tile_pool(name="st", bufs=2, space="PSUM"))
    # q-layout PV output: [128 q, 4 tiles, 65] fp32 = 1040B -> one bank.
    o_psum = ctx.enter_context(tc.tile_pool(name="opv", bufs=1, space="PSUM"))
    # X^T transpose staging ([128,512] fp32) and proj output y ([128,2,256]
    # fp32) are both 2KB/partition and share the last psum bank.
    oy_psum = ctx.enter_context(tc.tile_pool(name="oy", bufs=1, space="PSUM"))

    # W^T DMA is emitted lazily (first emit_proj) so the first batch's q/k
    # loads go ahead of it on the sync DMA queue and PE starts sooner.
    wt_state = {}

    def get_wt():
        if "wt" not in wt_state:
            wt_sb = consts.tile([P, 2, E], TD)
            nc.sync.dma_start(wt_sb[:], wT.rearrange("(c p) e -> p c e", p=P))
            wt_state["wt"] = wt_sb
        return wt_state["wt"]

    zero_reg = nc.gpsimd.to_reg(0.0)

    # 128x128 bf16 identity for PE transpose-mode
    ident = consts.tile([P, P], BF, tag="ident")
    nc.gpsimd.memset(ident[:], 1.0)
    nc.gpsimd.affine_select(
        out=ident[:], in_=ident[:], compare_op=OP.is_equal, fill=zero_reg,
        base=0, pattern=[[1, P]], channel_multiplier=-1,
    )

    # Warm-up: a 1-column matmul with no DMA dependency absorbs the
    # first-matmul pipeline/dispatch cost while the q/k loads are in flight.
    warm = st_psum.tile([P, ST_COLS], FP, tag="st", name="warmup")
    nc.tensor.matmul(
        warm[0:1, 0:1], lhsT=ident[:, 0:1], rhs=ident[:, 0:1], start=True,
        stop=True,
    )

    # ---- software-pipelined emission over (batch, head) units ----
    # Emitting S^T(h+1) before the exp/PV/normalize tail of h keeps the
    # in-order PE queue free of head-of-line stalls (PE always has the next
    # head's QK matmuls ready while ACT/DVE drain the previous head).
    batch_states = {}

    def emit_head(bu, rep):
        b, h = bu
        if h == 0:
            # q/k loads split by E-half: heads 0/1 only need the c=0 half,
            # so their QK starts after half the load bytes
            qt_sb = qk_pool.tile([P, 2, S], TD, tag="qt", name=f"qt{b}_{rep}")
            kt_sb = qk_pool.tile([P, 2, S], TD, tag="kt", name=f"kt{b}_{rep}")
            qT_r = qT[b].rearrange("(c p) s -> p c s", p=P)
            kT_r = kT[b].rearrange("(c p) s -> p c s", p=P)
            for c in range(2):
                nc.sync.dma_start(qt_sb[:, c, :], qT_r[:, c, :])
                nc.sync.dma_start(kt_sb[:, c, :], kT_r[:, c, :])
            # V loads ride the sync queue (Pool carries the affine_select
            # masks and is the busier of the two); stores stay on Pool
            va = v_pool.tile([P, 4, H, P], TD, tag="va", name=f"va{b}_{rep}")
            nc.gpsimd.memset(va[:, :, :, D:], 1.0)
            for j in range(4):
                nc.sync.dma_start(
                    va[:, j, :, :D],
                    v[b, P * j : P * (j + 1)].rearrange("p (h d) -> p h d", d=D),
                )
            xt_sb = xt_pool.tile([P, 2, S], TD, tag="xt", name=f"xt{b}_{rep}")
            batch_states[b] = (qt_sb, kt_sb, va, xt_sb)
        qt_sb, kt_sb, va, xt_sb = batch_states[b]

        hp = D * (h % 2)
        hc = h // 2
        st = st_psum.tile([P, ST_COLS], FP, tag="st", name=f"st{b}_{h}_{rep}")
        for j in range(4):
            qoff = P * j
            qr = S - qoff
            col = CHUNK_COL[j]
            nc.tensor.matmul(
                st[:, col : col + qr],
                lhsT=kt_sb[hp : hp + D, hc, qoff : qoff + P],
                rhs=qt_sb[hp : hp + D, hc, qoff:S],
                start=True,
                stop=True,
            )
        return (b, h, st, va, xt_sb)

    def emit_tail(state, rep):
        b, h, st, va, xt_sb = state
        hp = D * (h % 2)
        hc = h // 2
        pt = pt_pool.tile([P, ST_COLS], TD, tag="pt", name=f"pt{b}_{h}_{rep}")
        nc.scalar.activation(pt[:], st[:, 0:ST_COLS], AF.Exp, scale=0.125)
        # zero the causal triangle of each 128x128 diagonal block:
        # keep iff q_rel - k >= 0 (k = partition, q_rel = col in block)
        for j in range(4):
            col = CHUNK_COL[j]
            nc.gpsimd.affine_select(
                out=pt[:, col : col + P],
                in_=pt[:, col : col + P],
                compare_op=OP.is_ge,
                fill=zero_reg,
                base=0,
                pattern=[[1, P]],
                channel_multiplier=-1,
            )

        # PV in q-partition layout: per q-tile i, accumulate key-chunks
        # j<=i with lhsT = the q-tile slice of P^T and rhs = [V_h | one
        # ones-column] -> O[q, 0:64] plus the softmax denominator in col 64.
        opv = o_psum.tile([P, 4, 65], FP, tag="opv", name=f"o{b}_{h}_{rep}")
        for i in range(4):
            for j in range(i + 1):
                col = CHUNK_COL[j] + P * (i - j)
                nc.tensor.matmul(
                    opv[:, i, :],
                    lhsT=pt[:, col : col + P],
                    rhs=va[:, j, h, 0:65],
                    start=(j == 0),
                    stop=(j == i),
                )

        return (b, h, opv, xt_sb)

    pair_xt = {}

    def emit_norm(nstate, rep, borrow_st=False):
        b, h, opv, xt_sb = nstate
        hp = D * (h % 2)
        hc = h // 2
        # normalize in q-layout: the denominator is one column per q-tile, so
        # the reciprocal runs on 4 elements/partition instead of 512, and the
        # divide is one broadcast multiply. Then 4 PE transposes restore the
        # X^T layout the projection needs (psum staged through the shared
        # oy bank, evacuated once per head pair).
        r4 = r_pool.tile([P, 4], FP, tag="rsb", name=f"r{b}_{h}_{rep}")
        nc.vector.reciprocal_approx_fast(r4[:], opv[:, :, 64])
        # normalize into the PAIR tile: head h fills columns [hp, hp+64) of
        # each q-tile block. At h odd, FOUR [128,128] transposes handle the
        # whole pair (the transpose's cost is its 128 output columns either
        # way -- stacking both heads' d-halves on the partition dim halves
        # the transpose count, cutting ~3.4us off the PE wall).
        if h % 2 == 0:
            pair_xt["x"] = r_pool.tile(
                [P, 4, P], BF, tag="xsb", name=f"x{b}_{hc}_{rep}"
            )
        x_pair = pair_xt["x"]
        nc.vector.tensor_tensor(
            x_pair[:, :, hp : hp + D],
            opv[:, :, 0:D],
            r4[:].unsqueeze(2).broadcast_to((P, 4, D)),
            OP.mult,
        )
        if h % 2 == 1:
            if borrow_st:
                # drain time: the st banks are idle, so the last pair's
                # transpose staging borrows one instead of waiting for the
                # previous pair's evacuation of the shared oy bank
                xst = st_psum.tile([P, ST_COLS], FP, tag="st", name=f"xb{b}_{rep}")
                xt_ps = xst[:, 0:512].bitcast(BF).rearrange(
                    "p (two i c) -> p two i c", two=2, c=P
                )
            else:
                # bf16 transpose staging padded to 2KB so it tag-shares the
                # single oy psum bank with the fp32 projection output
                xt_ps = oy_psum.tile(
                    [P, 2, 4, P], BF, tag="oy", name=f"xp{b}_{hc}_{rep}"
                )
            for i in range(4):
                nc.tensor.transpose(
                    xt_ps[:, 0, i, :], x_pair[:, i, :], ident[:]
                )
            nc.vector.tensor_copy(xt_sb[:, hc, :], xt_ps[:, 0, :, :])

        return (b, xt_sb) if h == H - 1 else None

    proj_seq = {"n": 0, "total": None}

    def emit_proj(pstate, rep):
        b, xt_sb = pstate
        proj_seq["n"] += 1
        last = proj_seq["n"] == proj_seq["total"]
        # out-projection, two 1-bank psum halves; egress split across the
        # scalar and vector engines (bias is added host-side)
        yout = yo_pool.tile([P, 4, E], BF, tag="yout", name=f"yo{b}_{rep}")
        # ONE psum slot serves both projection halves (qp1 overwrites after
        # qp0's egress) so the second oy slot stays free and the next batch's
        # PV never waits on this batch's y egress -- that round-trip was a
        # 1.8us DVE stall at every batch boundary. The LAST projection has
        # nothing behind it, so it takes both slots and splits its egress
        # across the scalar and vector engines to shorten the drain.
        y = oy_psum.tile([P, 2, E], FP, tag="oy", name=f"y{b}_{rep}")
        for qp in range(2):
            if last and qp == 1:
                # the st banks are idle during the drain: borrow one so the
                # final qp1 projection never waits on qp0's egress copy
                y_st = st_psum.tile([P, ST_COLS], FP, tag="st", name=f"y{b}b_{rep}")
                y = y_st[:, 0 : 2 * E].rearrange("p (c e) -> p c e", c=2)
            for qi in range(2):
                qt_idx = 2 * qp + qi
                for c in range(2):
                    nc.tensor.matmul(
                        y[:, qi, :],
                        lhsT=xt_sb[:, c, P * qt_idx : P * (qt_idx + 1)],
                        rhs=get_wt()[:, c, :],
                        start=(c == 0),
                        stop=(c == 1),
                    )
            if last and qp == 0:
                nc.scalar.copy(yout[:, 0:2, :], y[:])
            else:
                nc.vector.tensor_copy(yout[:, 2 * qp : 2 * qp + 2, :], y[:])
            if last:
                # split the final store so the first half flies while the
                # second egress copy is still running
                nc.gpsimd.dma_start(
                    out[b].rearrange("(g p) e -> p g e", p=P)[
                        :, 2 * qp : 2 * qp + 2, :
                    ],
                    yout[:, 2 * qp : 2 * qp + 2, :],
                )
        if not last:
            nc.gpsimd.dma_start(
                out[b].rearrange("(g p) e -> p g e", p=P), yout[:]
            )
        del batch_states[b]

    # Software pipeline, 3 stages deep: heads (QK) run 2 units ahead of the
    # exp/PV front-tails, and each head's normalize trails its PV by one more
    # unit. The PE queue always has two heads of QK ready, and every
    # normalize op's dependencies are settled before it is dequeued.
    units = [(b, h, rep) for rep in range(repeat) for b in range(BPC) for h in range(H)]
    proj_seq["total"] = BPC * repeat
    heads = []  # states awaiting their front-tail (2 units behind)
    norms = []  # states awaiting their normalize (1 further unit behind)
    projs = []  # batches awaiting their projection
    for unit in units:
        heads.append((emit_head(unit[:2], unit[2]), unit[2]))
        if len(heads) > 2:
            state, trep = heads.pop(0)
            norms.append((emit_tail(state, trep), trep))
        if len(norms) > 1:
            nstate, trep = norms.pop(0)
            pstate = emit_norm(nstate, trep)
            if pstate is not None:
                projs.append((pstate, trep))
            elif projs:
                emit_proj(*projs.pop(0))
    while heads or norms or projs:
        if heads:
            state, trep = heads.pop(0)
            norms.append((emit_tail(state, trep), trep))
        if norms:
            nstate, trep = norms.pop(0)
            pstate = emit_norm(nstate, trep, borrow_st=True)
            if pstate is not None:
                projs.append((pstate, trep))
                continue
        if projs:
            emit_proj(*projs.pop(0))


def build_nc(bpc=BPC, repeat=1):
    from contextlib import ExitStack

    nc = bacc.Bacc("TRN2", target_bir_lowering=False, debug=False)
    qT = nc.dram_tensor("qT", [bpc, E, S], TD, kind="ExternalInput").ap()
    kT = nc.dram_tensor("kT", [bpc, E, S], TD, kind="ExternalInput").ap()
    v = nc.dram_tensor("v", [bpc, S, E], TD, kind="ExternalInput").ap()
    wT = nc.dram_tensor("wT", [E, E], TD, kind="ExternalInput").ap()
    bo = nc.dram_tensor("bo", [1, E], FP, kind="ExternalInput").ap()
    out = nc.dram_tensor("out", [bpc, S, E], BF, kind="ExternalOutput").ap()

    with tile.TileContext(nc) as tc:
        with ExitStack() as ctx:
            saved = globals()["BPC"]
            globals()["BPC"] = bpc
            try:
                attention_kernel(ctx, tc, out, qT, kT, v, wT, bo, repeat=repeat)
            finally:
                globals()["BPC"] = saved
    nc.compile()
    return nc


def _np_td():
    import ml_dtypes

    return np.dtype(ml_dtypes.bfloat16)


def make_in_maps(qry, key, val, w_out, b_out):
    td = _np_td()
    qT_all = np.ascontiguousarray(qry.transpose(0, 2, 1)).astype(td)
    kT_all = np.ascontiguousarray(key.transpose(0, 2, 1)).astype(td)
    val = val.astype(td)
    wT = np.ascontiguousarray(w_out.T).astype(td)
    bo = np.ascontiguousarray(b_out.reshape(1, E), dtype=np.float32)
    maps = []
    for c in range(NCORES):
        sl = slice(c * BPC, (c + 1) * BPC)
        maps.append(
            {
                "qT": qT_all[sl],
                "kT": kT_all[sl],
                "v": np.ascontiguousarray(val[sl]),
                "wT": wT,
                "bo": bo,
            }
        )
    return maps


_NC_CACHE = {}


def _get_nc():
    if "nc" not in _NC_CACHE:
        _NC_CACHE["nc"] = build_nc()
    return _NC_CACHE["nc"]


def kernel(qry, key, val, w_out, b_out, **run_kwargs):
    nc = _get_nc()
    in_maps = make_in_maps(
        np.asarray(qry, dtype=np.float32),
        np.asarray(key, dtype=np.float32),
        np.asarray(val, dtype=np.float32),
        np.asarray(w_out, dtype=np.float32),
        np.asarray(b_out, dtype=np.float32),
    )
    res = run_bass_kernel_spmd(nc, in_maps, core_ids=list(range(NCORES)), **run_kwargs)
    out = np.concatenate(
        [res.results[c]["out"].astype(np.float32) for c in range(NCORES)], axis=0
    )
    out += np.asarray(b_out, dtype=np.float32)
    if run_kwargs:
        kernel.last_results = res
    return out



# revision 8
# speedup vs baseline: 1.0033x; 1.0033x over previous
"""Causal MHA + out-projection Trainium2 kernel (B=64, S=512, E=256, H=4).

Sharding: data-parallel over batch, 8 batches per NeuronCore x 8 cores.

Per (batch, head):
  - S^T[k, q] = sum_d K[k,d] Q[q,d]  (chunks of 128 keys; q free dim,
    causally restricted to q >= 128*j for key-chunk j)
  - P^T = exp(S^T/8) on the scalar engine (one op per head); the causal
    triangle inside each 128x128 diagonal block is zeroed afterwards with
    one gpsimd affine_select per chunk (cheaper than mask matmuls on PE).
  - PV in q-partition layout: per q-tile i, key-chunks j<=i accumulate
    matmul(lhsT = q-tile slice of P^T, rhs = [V_h | one ones-column]) into
    O[128q, 4, 65] -- a single PSUM bank, with the softmax denominator
    landing as one per-partition column per q-tile.
  - Normalize is therefore tiny on the vector engine: a 4-column
    reciprocal_approx_fast reading the denominator column straight from
    PSUM, and ONE broadcast multiply (stride-0 free dim) writing this
    head's 64-column half of a PAIR tile X[q, 4 tiles, 128] in bf16.
  - Four [128,128] PE transpose-mode matmuls per head PAIR (not per head:
    a transpose costs its 128 output columns regardless of partition
    count, so stacking both heads' d-halves on the partition dim halves
    the transpose count) restore X^T[d-pair, q] into bf16 PSUM staging
    tag-shared with the projection output, evacuated once per pair.
  - Y[q,:] = X^T.T @ W^T in one reused 1-bank PSUM slot, egressed as bf16 on
    the vector engine; the bias row is added on the host after the gather
    (a per-feature constant -- exact up to the bf16 output rounding).

The emission is software-pipelined 3 stages deep (QK heads 2 units ahead of
the exp/PV tails, normalize one further unit behind) so every op's
dependencies are settled before it reaches the front of its strict-FIFO
engine queue. Host-side prep: qry/key are pre-transposed to [B, E, S] and
w_out to W^T so that every device DMA is contiguous.
"""

import sys

if "/opt/trn_rl_repo" not in sys.path:
    sys.path.insert(0, "/opt/trn_rl_repo")

import numpy as np

import concourse.bass as bass
import concourse.mybir as mybir
import concourse.tile as tile
from concourse import bacc
from concourse.bass_utils import run_bass_kernel_spmd

B, S, E, H = 64, 512, 256, 4
D = E // H  # 64
P = 128
NCORES = 8
BPC = B // NCORES  # 8

FP = mybir.dt.float32
BF = mybir.dt.bfloat16
TD = BF

# S^T chunk -> column offset inside the 3-bank (1280 col) psum tile.
# chunk j covers keys [128j, 128j+128), q in [128j, 512) => width 512-128j.
CHUNK_COL = [0, 512, 1024, 896]
ST_COLS = 1280  # exp() span


def attention_kernel(ctx, tc, out, qT, kT, v, wT, bo, repeat=1):
    nc = tc.nc
    AF = mybir.ActivationFunctionType
    OP = mybir.AluOpType

    consts = ctx.enter_context(tc.tile_pool(name="consts", bufs=1))
    qk_pool = ctx.enter_context(tc.tile_pool(name="qk", bufs=3))
    v_pool = ctx.enter_context(tc.tile_pool(name="v", bufs=3))
    pt_pool = ctx.enter_context(tc.tile_pool(name="pt", bufs=3))
    xt_pool = ctx.enter_context(tc.tile_pool(name="xt", bufs=2))
    yo_pool = ctx.enter_context(tc.tile_pool(name="yo", bufs=2))
    r_pool = ctx.enter_context(tc.tile_pool(name="rsb", bufs=3))
    st_psum = ctx.enter_context(tc.tile_pool(name="st", bufs=2, space="PSUM"))
    # q-layout PV output: [128 q, 4 tiles, 65] fp32 = 1040B -> one bank.
    o_psum = ctx.enter_context(tc.tile_pool(name="opv", bufs=1, space="PSUM"))
    # X^T transpose staging ([128,512] fp32) and proj output y ([128,2,256]
    # fp32) are both 2KB/partition and share the last psum bank.
    oy_psum = ctx.enter_context(tc.tile_pool(name="oy", bufs=1, space="PSUM"))

    # W^T DMA is emitted lazily (first emit_proj) so the first batch's q/k
    # loads go ahead of it on the sync DMA queue and PE starts sooner.
    wt_state = {}

    def get_wt():
        if "wt" not in wt_state:
            wt_sb = consts.tile([P, 2, E], TD)
            nc.sync.dma_start(wt_sb[:], wT.rearrange("(c p) e -> p c e", p=P))
            wt_state["wt"] = wt_sb
        return wt_state["wt"]

    zero_reg = nc.gpsimd.to_reg(0.0)

    # 128x128 bf16 identity for PE transpose-mode
    ident = consts.tile([P, P], BF, tag="ident")
    nc.gpsimd.memset(ident[:], 1.0)
    nc.gpsimd.affine_select(
        out=ident[:], in_=ident[:], compare_op=OP.is_equal, fill=zero_reg,
        base=0, pattern=[[1, P]], channel_multiplier=-1,
    )

    # Warm-up: a 1-column matmul with no DMA dependency absorbs the
    # first-matmul pipeline/dispatch cost while the q/k loads are in flight.
    warm = st_psum.tile([P, ST_COLS], FP, tag="st", name="warmup")
    nc.tensor.matmul(
        warm[0:1, 0:1], lhsT=ident[:, 0:1], rhs=ident[:, 0:1], start=True,
        stop=True,
    )

    # ---- software-pipelined emission over (batch, head) units ----
    # Emitting S^T(h+1) before the exp/PV/normalize tail of h keeps the
    # in-order PE queue free of head-of-line stalls (PE always has the next
    # head's QK matmuls ready while ACT/DVE drain the previous head).
    batch_states = {}

    def emit_head(bu, rep):
        b, h = bu
        if h == 0:
            # q/k loads split by E-half: heads 0/1 only need the c=0 half,
            # so their QK starts after half the load bytes
            qt_sb = qk_pool.tile([P, 2, S], TD, tag="qt", name=f"qt{b}_{rep}")
            kt_sb = qk_pool.tile([P, 2, S], TD, tag="kt", name=f"kt{b}_{rep}")
            qT_r = qT[b].rearrange("(c p) s -> p c s", p=P)
            kT_r = kT[b].rearrange("(c p) s -> p c s", p=P)
            for c in range(2):
                nc.sync.dma_start(qt_sb[:, c, :], qT_r[:, c, :])
                nc.sync.dma_start(kt_sb[:, c, :], kT_r[:, c, :])
            # V loads ride the sync queue (Pool carries the affine_select
            # masks and is the busier of the two); stores stay on Pool
            va = v_pool.tile([P, 4, H, P], TD, tag="va", name=f"va{b}_{rep}")
            nc.gpsimd.memset(va[:, :, :, D:], 1.0)
            for j in range(4):
                nc.sync.dma_start(
                    va[:, j, :, :D],
                    v[b, P * j : P * (j + 1)].rearrange("p (h d) -> p h d", d=D),
                )
            xt_sb = xt_pool.tile([P, 2, S], TD, tag="xt", name=f"xt{b}_{rep}")
            batch_states[b] = (qt_sb, kt_sb, va, xt_sb)
        qt_sb, kt_sb, va, xt_sb = batch_states[b]

        hp = D * (h % 2)
        hc = h // 2
        st = st_psum.tile([P, ST_COLS], FP, tag="st", name=f"st{b}_{h}_{rep}")
        for j in range(4):
            qoff = P * j
            qr = S - qoff
            col = CHUNK_COL[j]
            nc.tensor.matmul(
                st[:, col : col + qr],
                lhsT=kt_sb[hp : hp + D, hc, qoff : qoff + P],
                rhs=qt_sb[hp : hp + D, hc, qoff:S],
                start=True,
                stop=True,
            )
        return (b, h, st, va, xt_sb)

    def emit_tail(state, rep):
        b, h, st, va, xt_sb = state
        hp = D * (h % 2)
        hc = h // 2
        pt = pt_pool.tile([P, ST_COLS], TD, tag="pt", name=f"pt{b}_{h}_{rep}")
        nc.scalar.activation(pt[:], st[:, 0:ST_COLS], AF.Exp, scale=0.125)
        # zero the causal triangle of each 128x128 diagonal block:
        # keep iff q_rel - k >= 0 (k = partition, q_rel = col in block)
        for j in range(4):
            col = CHUNK_COL[j]
            nc.gpsimd.affine_select(
                out=pt[:, col : col + P],
                in_=pt[:, col : col + P],
                compare_op=OP.is_ge,
                fill=zero_reg,
                base=0,
                pattern=[[1, P]],
                channel_multiplier=-1,
            )

        # PV in q-partition layout: per q-tile i, accumulate key-chunks
        # j<=i with lhsT = the q-tile slice of P^T and rhs = [V_h | one
        # ones-column] -> O[q, 0:64] plus the softmax denominator in col 64.
        opv = o_psum.tile([P, 4, 65], FP, tag="opv", name=f"o{b}_{h}_{rep}")
        for i in range(4):
            for j in range(i + 1):
                col = CHUNK_COL[j] + P * (i - j)
                nc.tensor.matmul(
                    opv[:, i, :],
                    lhsT=pt[:, col : col + P],
                    rhs=va[:, j, h, 0:65],
                    start=(j == 0),
                    stop=(j == i),
                )

        return (b, h, opv, xt_sb)

    pair_xt = {}

    def emit_norm(nstate, rep, borrow_st=False):
        b, h, opv, xt_sb = nstate
        hp = D * (h % 2)
        hc = h // 2
        # normalize in q-layout: the denominator is one column per q-tile, so
        # the reciprocal runs on 4 elements/partition instead of 512, and the
        # divide is one broadcast multiply. Then 4 PE transposes restore the
        # X^T layout the projection needs (psum staged through the shared
        # oy bank, evacuated once per head pair).
        r4 = r_pool.tile([P, 4], FP, tag="rsb", name=f"r{b}_{h}_{rep}")
        nc.vector.reciprocal_approx_fast(r4[:], opv[:, :, 64])
        # normalize into the PAIR tile: head h fills columns [hp, hp+64) of
        # each q-tile block. At h odd, FOUR [128,128] transposes handle the
        # whole pair (the transpose's cost is its 128 output columns either
        # way -- stacking both heads' d-halves on the partition dim halves
        # the transpose count, cutting ~3.4us off the PE wall).
        if h % 2 == 0:
            pair_xt["x"] = r_pool.tile(
                [P, 4, P], BF, tag="xsb", name=f"x{b}_{hc}_{rep}"
            )
        x_pair = pair_xt["x"]
        nc.vector.tensor_tensor(
            x_pair[:, :, hp : hp + D],
            opv[:, :, 0:D],
            r4[:].unsqueeze(2).broadcast_to((P, 4, D)),
            OP.mult,
        )
        if h % 2 == 1:
            if borrow_st:
                # drain time: the st banks are idle, so the last pair's
                # transpose staging borrows one instead of waiting for the
                # previous pair's evacuation of the shared oy bank
                xst = st_psum.tile([P, ST_COLS], FP, tag="st", name=f"xb{b}_{rep}")
                xt_ps = xst[:, 0:512].bitcast(BF).rearrange(
                    "p (two i c) -> p two i c", two=2, c=P
                )
            else:
                # bf16 transpose staging padded to 2KB so it tag-shares the
                # single oy psum bank with the fp32 projection output
                xt_ps = oy_psum.tile(
                    [P, 2, 4, P], BF, tag="oy", name=f"xp{b}_{hc}_{rep}"
                )
            for i in range(4):
                nc.tensor.transpose(
                    xt_ps[:, 0, i, :], x_pair[:, i, :], ident[:]
                )
            if borrow_st:
                # drain: ACT is idle after the final exp; evacuate there so
                # DVE's normalize of the last head isn't serialized with it
                nc.scalar.copy(xt_sb[:, hc, :], xt_ps[:, 0, :, :])
            else:
                nc.vector.tensor_copy(xt_sb[:, hc, :], xt_ps[:, 0, :, :])

        return (b, xt_sb) if h == H - 1 else None

    proj_seq = {"n": 0, "total": None}

    def emit_proj(pstate, rep):
        b, xt_sb = pstate
        proj_seq["n"] += 1
        last = proj_seq["n"] == proj_seq["total"]
        # out-projection, two 1-bank psum halves; egress split across the
        # scalar and vector engines (bias is added host-side)
        yout = yo_pool.tile([P, 4, E], BF, tag="yout", name=f"yo{b}_{rep}")
        # ONE psum slot serves both projection halves (qp1 overwrites after
        # qp0's egress) so the second oy slot stays free and the next batch's
        # PV never waits on this batch's y egress -- that round-trip was a
        # 1.8us DVE stall at every batch boundary. The LAST projection has
        # nothing behind it, so it takes both slots and splits its egress
        # across the scalar and vector engines to shorten the drain.
        y = oy_psum.tile([P, 2, E], FP, tag="oy", name=f"y{b}_{rep}")
        for qp in range(2):
            if last and qp == 1:
                # the st banks are idle during the drain: borrow one so the
                # final qp1 projection never waits on qp0's egress copy
                y_st = st_psum.tile([P, ST_COLS], FP, tag="st", name=f"y{b}b_{rep}")
                y = y_st[:, 0 : 2 * E].rearrange("p (c e) -> p c e", c=2)
            for qi in range(2):
                qt_idx = 2 * qp + qi
                for c in range(2):
                    nc.tensor.matmul(
                        y[:, qi, :],
                        lhsT=xt_sb[:, c, P * qt_idx : P * (qt_idx + 1)],
                        rhs=get_wt()[:, c, :],
                        start=(c == 0),
                        stop=(c == 1),
                    )
            if last and qp == 0:
                nc.scalar.copy(yout[:, 0:2, :], y[:])
            else:
                nc.vector.tensor_copy(yout[:, 2 * qp : 2 * qp + 2, :], y[:])
            if last:
                # split the final store so the first half flies while the
                # second egress copy is still running
                nc.gpsimd.dma_start(
                    out[b].rearrange("(g p) e -> p g e", p=P)[
                        :, 2 * qp : 2 * qp + 2, :
                    ],
                    yout[:, 2 * qp : 2 * qp + 2, :],
                )
        if not last:
            nc.gpsimd.dma_start(
                out[b].rearrange("(g p) e -> p g e", p=P), yout[:]
            )
        del batch_states[b]

    # Software pipeline, 3 stages deep: heads (QK) run 2 units ahead of the
    # exp/PV front-tails, and each head's normalize trails its PV by one more
    # unit. The PE queue always has two heads of QK ready, and every
    # normalize op's dependencies are settled before it is dequeued.
    units = [(b, h, rep) for rep in range(repeat) for b in range(BPC) for h in range(H)]
    proj_seq["total"] = BPC * repeat
    heads = []  # states awaiting their front-tail (2 units behind)
    norms = []  # states awaiting their normalize (1 further unit behind)
    projs = []  # batches awaiting their projection
    for unit in units:
        heads.append((emit_head(unit[:2], unit[2]), unit[2]))
        if len(heads) > 2:
            state, trep = heads.pop(0)
            norms.append((emit_tail(state, trep), trep))
        if len(norms) > 1:
            nstate, trep = norms.pop(0)
            pstate = emit_norm(nstate, trep)
            if pstate is not None:
                projs.append((pstate, trep))
            elif projs:
                emit_proj(*projs.pop(0))
    while heads or norms or projs:
        if heads:
            state, trep = heads.pop(0)
            norms.append((emit_tail(state, trep), trep))
        if norms:
            nstate, trep = norms.pop(0)
            pstate = emit_norm(nstate, trep, borrow_st=True)
            if pstate is not None:
                projs.append((pstate, trep))
                continue
        if projs:
            emit_proj(*projs.pop(0))


def build_nc(bpc=BPC, repeat=1):
    from contextlib import ExitStack

    nc = bacc.Bacc("TRN2", target_bir_lowering=False, debug=False)
    qT = nc.dram_tensor("qT", [bpc, E, S], TD, kind="ExternalInput").ap()
    kT = nc.dram_tensor("kT", [bpc, E, S], TD, kind="ExternalInput").ap()
    v = nc.dram_tensor("v", [bpc, S, E], TD, kind="ExternalInput").ap()
    wT = nc.dram_tensor("wT", [E, E], TD, kind="ExternalInput").ap()
    bo = nc.dram_tensor("bo", [1, E], FP, kind="ExternalInput").ap()
    out = nc.dram_tensor("out", [bpc, S, E], BF, kind="ExternalOutput").ap()

    with tile.TileContext(nc) as tc:
        with ExitStack() as ctx:
            saved = globals()["BPC"]
            globals()["BPC"] = bpc
            try:
                attention_kernel(ctx, tc, out, qT, kT, v, wT, bo, repeat=repeat)
            finally:
                globals()["BPC"] = saved
    nc.compile()
    return nc


def _np_td():
    import ml_dtypes

    return np.dtype(ml_dtypes.bfloat16)


def make_in_maps(qry, key, val, w_out, b_out):
    td = _np_td()
    qT_all = np.ascontiguousarray(qry.transpose(0, 2, 1)).astype(td)
    kT_all = np.ascontiguousarray(key.transpose(0, 2, 1)).astype(td)
    val = val.astype(td)
    wT = np.ascontiguousarray(w_out.T).astype(td)
    bo = np.ascontiguousarray(b_out.reshape(1, E), dtype=np.float32)
    maps = []
    for c in range(NCORES):
        sl = slice(c * BPC, (c + 1) * BPC)
        maps.append(
            {
                "qT": qT_all[sl],
                "kT": kT_all[sl],
                "v": np.ascontiguousarray(val[sl]),
                "wT": wT,
                "bo": bo,
            }
        )
    return maps


_NC_CACHE = {}


def _get_nc():
    if "nc" not in _NC_CACHE:
        _NC_CACHE["nc"] = build_nc()
    return _NC_CACHE["nc"]


def kernel(qry, key, val, w_out, b_out, **run_kwargs):
    nc = _get_nc()
    in_maps = make_in_maps(
        np.asarray(qry, dtype=np.float32),
        np.asarray(key, dtype=np.float32),
        np.asarray(val, dtype=np.float32),
        np.asarray(w_out, dtype=np.float32),
        np.asarray(b_out, dtype=np.float32),
    )
    res = run_bass_kernel_spmd(nc, in_maps, core_ids=list(range(NCORES)), **run_kwargs)
    out = np.concatenate(
        [res.results[c]["out"].astype(np.float32) for c in range(NCORES)], axis=0
    )
    out += np.asarray(b_out, dtype=np.float32)
    if run_kwargs:
        kernel.last_results = res
    return out



# revision 9
# speedup vs baseline: 1.0074x; 1.0041x over previous
"""Causal MHA + out-projection Trainium2 kernel (B=64, S=512, E=256, H=4).

Sharding: data-parallel over batch, 8 batches per NeuronCore x 8 cores.

Per (batch, head):
  - S^T[k, q] = sum_d K[k,d] Q[q,d]  (chunks of 128 keys; q free dim,
    causally restricted to q >= 128*j for key-chunk j)
  - P^T = exp(S^T/8) on the scalar engine (one op per head); the causal
    triangle inside each 128x128 diagonal block is zeroed afterwards with
    one gpsimd affine_select per chunk (cheaper than mask matmuls on PE).
  - PV in q-partition layout: per q-tile i, key-chunks j<=i accumulate
    matmul(lhsT = q-tile slice of P^T, rhs = [V_h | one ones-column]) into
    O[128q, 4, 65] -- a single PSUM bank, with the softmax denominator
    landing as one per-partition column per q-tile.
  - Normalize is therefore tiny on the vector engine: a 4-column
    reciprocal_approx_fast reading the denominator column straight from
    PSUM, and ONE broadcast multiply (stride-0 free dim) writing this
    head's 64-column half of a PAIR tile X[q, 4 tiles, 128] in bf16.
  - Four [128,128] PE transpose-mode matmuls per head PAIR (not per head:
    a transpose costs its 128 output columns regardless of partition
    count, so stacking both heads' d-halves on the partition dim halves
    the transpose count) restore X^T[d-pair, q] into bf16 PSUM staging
    tag-shared with the projection output, evacuated once per pair.
  - Y[q,:] = X^T.T @ W^T in one reused 1-bank PSUM slot, egressed as bf16 on
    the vector engine; the bias row is added on the host after the gather
    (a per-feature constant -- exact up to the bf16 output rounding).

The emission is software-pipelined 3 stages deep (QK heads 2 units ahead of
the exp/PV tails, normalize one further unit behind) so every op's
dependencies are settled before it reaches the front of its strict-FIFO
engine queue. Host-side prep: qry/key are pre-transposed to [B, E, S] and
w_out to W^T so that every device DMA is contiguous.
"""

import sys

if "/opt/trn_rl_repo" not in sys.path:
    sys.path.insert(0, "/opt/trn_rl_repo")

import numpy as np

import concourse.bass as bass
import concourse.mybir as mybir
import concourse.tile as tile
from concourse import bacc
from concourse.bass_utils import run_bass_kernel_spmd

B, S, E, H = 64, 512, 256, 4
D = E // H  # 64
P = 128
NCORES = 8
BPC = B // NCORES  # 8

FP = mybir.dt.float32
BF = mybir.dt.bfloat16
TD = BF

# S^T chunk -> column offset inside the 3-bank (1280 col) psum tile.
# chunk j covers keys [128j, 128j+128), q in [128j, 512) => width 512-128j.
CHUNK_COL = [0, 512, 1024, 896]
ST_COLS = 1280  # exp() span


def attention_kernel(ctx, tc, out, qT, kT, v, wT, bo, repeat=1):
    nc = tc.nc
    AF = mybir.ActivationFunctionType
    OP = mybir.AluOpType

    consts = ctx.enter_context(tc.tile_pool(name="consts", bufs=1))
    qk_pool = ctx.enter_context(tc.tile_pool(name="qk", bufs=3))
    v_pool = ctx.enter_context(tc.tile_pool(name="v", bufs=3))
    pt_pool = ctx.enter_context(tc.tile_pool(name="pt", bufs=3))
    xt_pool = ctx.enter_context(tc.tile_pool(name="xt", bufs=2))
    yo_pool = ctx.enter_context(tc.tile_pool(name="yo", bufs=2))
    r_pool = ctx.enter_context(tc.tile_pool(name="rsb", bufs=3))
    st_psum = ctx.enter_context(tc.tile_pool(name="st", bufs=2, space="PSUM"))
    # q-layout PV output: [128 q, 4 tiles, 65] fp32 = 1040B -> one bank.
    o_psum = ctx.enter_context(tc.tile_pool(name="opv", bufs=1, space="PSUM"))
    # X^T transpose staging ([128,512] fp32) and proj output y ([128,2,256]
    # fp32) are both 2KB/partition and share the last psum bank.
    oy_psum = ctx.enter_context(tc.tile_pool(name="oy", bufs=1, space="PSUM"))

    # W^T DMA is emitted lazily (first emit_proj) so the first batch's q/k
    # loads go ahead of it on the sync DMA queue and PE starts sooner.
    wt_state = {}

    def get_wt():
        if "wt" not in wt_state:
            wt_sb = consts.tile([P, 2, E], TD)
            nc.sync.dma_start(wt_sb[:], wT.rearrange("(c p) e -> p c e", p=P))
            wt_state["wt"] = wt_sb
        return wt_state["wt"]

    zero_reg = nc.gpsimd.to_reg(0.0)

    # 128x128 bf16 identity for PE transpose-mode
    ident = consts.tile([P, P], BF, tag="ident")
    nc.gpsimd.memset(ident[:], 1.0)
    nc.gpsimd.affine_select(
        out=ident[:], in_=ident[:], compare_op=OP.is_equal, fill=zero_reg,
        base=0, pattern=[[1, P]], channel_multiplier=-1,
    )

    # Warm-up: a 1-column matmul with no DMA dependency absorbs the
    # first-matmul pipeline/dispatch cost while the q/k loads are in flight.
    warm = st_psum.tile([P, ST_COLS], FP, tag="st", name="warmup")
    nc.tensor.matmul(
        warm[0:1, 0:1], lhsT=ident[:, 0:1], rhs=ident[:, 0:1], start=True,
        stop=True,
    )

    # ---- software-pipelined emission over (batch, head) units ----
    # Emitting S^T(h+1) before the exp/PV/normalize tail of h keeps the
    # in-order PE queue free of head-of-line stalls (PE always has the next
    # head's QK matmuls ready while ACT/DVE drain the previous head).
    batch_states = {}

    def emit_head(bu, rep):
        b, h = bu
        if h == 0:
            # q/k loads split by E-half: heads 0/1 only need the c=0 half,
            # so their QK starts after half the load bytes
            qt_sb = qk_pool.tile([P, 2, S], TD, tag="qt", name=f"qt{b}_{rep}")
            kt_sb = qk_pool.tile([P, 2, S], TD, tag="kt", name=f"kt{b}_{rep}")
            qT_r = qT[b].rearrange("(c p) s -> p c s", p=P)
            kT_r = kT[b].rearrange("(c p) s -> p c s", p=P)
            for c in range(2):
                nc.sync.dma_start(qt_sb[:, c, :], qT_r[:, c, :])
                nc.sync.dma_start(kt_sb[:, c, :], kT_r[:, c, :])
            # V loads ride the sync queue (Pool carries the affine_select
            # masks and is the busier of the two); stores stay on Pool
            va = v_pool.tile([P, 4, H, P], TD, tag="va", name=f"va{b}_{rep}")
            nc.gpsimd.memset(va[:, :, :, D:], 1.0)
            for j in range(4):
                nc.sync.dma_start(
                    va[:, j, :, :D],
                    v[b, P * j : P * (j + 1)].rearrange("p (h d) -> p h d", d=D),
                )
            xt_sb = xt_pool.tile([P, 2, S], TD, tag="xt", name=f"xt{b}_{rep}")
            batch_states[b] = (qt_sb, kt_sb, va, xt_sb)
        qt_sb, kt_sb, va, xt_sb = batch_states[b]

        hp = D * (h % 2)
        hc = h // 2
        st = st_psum.tile([P, ST_COLS], FP, tag="st", name=f"st{b}_{h}_{rep}")
        for j in range(4):
            qoff = P * j
            qr = S - qoff
            col = CHUNK_COL[j]
            nc.tensor.matmul(
                st[:, col : col + qr],
                lhsT=kt_sb[hp : hp + D, hc, qoff : qoff + P],
                rhs=qt_sb[hp : hp + D, hc, qoff:S],
                start=True,
                stop=True,
            )
        return (b, h, st, va, xt_sb)

    def emit_tail(state, rep):
        b, h, st, va, xt_sb = state
        hp = D * (h % 2)
        hc = h // 2
        pt = pt_pool.tile([P, ST_COLS], TD, tag="pt", name=f"pt{b}_{h}_{rep}")
        nc.scalar.activation(pt[:], st[:, 0:ST_COLS], AF.Exp, scale=0.125)
        # zero the causal triangle of each 128x128 diagonal block:
        # keep iff q_rel - k >= 0 (k = partition, q_rel = col in block)
        for j in range(4):
            col = CHUNK_COL[j]
            nc.gpsimd.affine_select(
                out=pt[:, col : col + P],
                in_=pt[:, col : col + P],
                compare_op=OP.is_ge,
                fill=zero_reg,
                base=0,
                pattern=[[1, P]],
                channel_multiplier=-1,
            )

        # PV in q-partition layout: per q-tile i, accumulate key-chunks
        # j<=i with lhsT = the q-tile slice of P^T and rhs = [V_h | one
        # ones-column] -> O[q, 0:64] plus the softmax denominator in col 64.
        opv = o_psum.tile([P, 4, 65], FP, tag="opv", name=f"o{b}_{h}_{rep}")
        for i in range(4):
            for j in range(i + 1):
                col = CHUNK_COL[j] + P * (i - j)
                nc.tensor.matmul(
                    opv[:, i, :],
                    lhsT=pt[:, col : col + P],
                    rhs=va[:, j, h, 0:65],
                    start=(j == 0),
                    stop=(j == i),
                )

        return (b, h, opv, xt_sb)

    pair_xt = {}

    def emit_norm(nstate, rep, borrow_st=False):
        b, h, opv, xt_sb = nstate
        hp = D * (h % 2)
        hc = h // 2
        # normalize in q-layout: the denominator is one column per q-tile, so
        # the reciprocal runs on 4 elements/partition instead of 512, and the
        # divide is one broadcast multiply. Then 4 PE transposes restore the
        # X^T layout the projection needs (psum staged through the shared
        # oy bank, evacuated once per head pair).
        r4 = r_pool.tile([P, 4], FP, tag="rsb", name=f"r{b}_{h}_{rep}")
        nc.vector.reciprocal_approx_fast(r4[:], opv[:, :, 64])
        # normalize into the PAIR tile: head h fills columns [hp, hp+64) of
        # each q-tile block. At h odd, FOUR [128,128] transposes handle the
        # whole pair (the transpose's cost is its 128 output columns either
        # way -- stacking both heads' d-halves on the partition dim halves
        # the transpose count, cutting ~3.4us off the PE wall).
        if h % 2 == 0:
            pair_xt["x"] = r_pool.tile(
                [P, 4, P], BF, tag="xsb", name=f"x{b}_{hc}_{rep}"
            )
        x_pair = pair_xt["x"]
        nc.vector.tensor_tensor(
            x_pair[:, :, hp : hp + D],
            opv[:, :, 0:D],
            r4[:].unsqueeze(2).broadcast_to((P, 4, D)),
            OP.mult,
        )
        if h % 2 == 1:
            if borrow_st:
                # drain time: the st banks are idle, so the last pair's
                # transpose staging borrows one instead of waiting for the
                # previous pair's evacuation of the shared oy bank
                xst = st_psum.tile([P, ST_COLS], FP, tag="st", name=f"xb{b}_{rep}")
                xt_ps = xst[:, 0:512].bitcast(BF).rearrange(
                    "p (two i c) -> p two i c", two=2, c=P
                )
            else:
                # bf16 transpose staging padded to 2KB so it tag-shares the
                # single oy psum bank with the fp32 projection output
                xt_ps = oy_psum.tile(
                    [P, 2, 4, P], BF, tag="oy", name=f"xp{b}_{hc}_{rep}"
                )
            for i in range(4):
                nc.tensor.transpose(
                    xt_ps[:, 0, i, :], x_pair[:, i, :], ident[:]
                )
            if borrow_st:
                # drain: ACT and DVE are both winding down; split the
                # evacuation across them so the final projections (gated on
                # this copy) start half an op earlier
                nc.scalar.copy(xt_sb[:, hc, 0:256], xt_ps[:, 0, 0:2, :])
                nc.vector.tensor_copy(xt_sb[:, hc, 256:512], xt_ps[:, 0, 2:4, :])
            else:
                nc.vector.tensor_copy(xt_sb[:, hc, :], xt_ps[:, 0, :, :])

        return (b, xt_sb) if h == H - 1 else None

    proj_seq = {"n": 0, "total": None}

    def emit_proj(pstate, rep):
        b, xt_sb = pstate
        proj_seq["n"] += 1
        last = proj_seq["n"] == proj_seq["total"]
        # out-projection, two 1-bank psum halves; egress split across the
        # scalar and vector engines (bias is added host-side)
        yout = yo_pool.tile([P, 4, E], BF, tag="yout", name=f"yo{b}_{rep}")
        # ONE psum slot serves both projection halves (qp1 overwrites after
        # qp0's egress) so the second oy slot stays free and the next batch's
        # PV never waits on this batch's y egress -- that round-trip was a
        # 1.8us DVE stall at every batch boundary. The LAST projection has
        # nothing behind it, so it takes both slots and splits its egress
        # across the scalar and vector engines to shorten the drain.
        y = oy_psum.tile([P, 2, E], FP, tag="oy", name=f"y{b}_{rep}")
        for qp in range(2):
            if last and qp == 1:
                # the st banks are idle during the drain: borrow one so the
                # final qp1 projection never waits on qp0's egress copy
                y_st = st_psum.tile([P, ST_COLS], FP, tag="st", name=f"y{b}b_{rep}")
                y = y_st[:, 0 : 2 * E].rearrange("p (c e) -> p c e", c=2)
            for qi in range(2):
                qt_idx = 2 * qp + qi
                for c in range(2):
                    nc.tensor.matmul(
                        y[:, qi, :],
                        lhsT=xt_sb[:, c, P * qt_idx : P * (qt_idx + 1)],
                        rhs=get_wt()[:, c, :],
                        start=(c == 0),
                        stop=(c == 1),
                    )
            if last and qp == 0:
                nc.scalar.copy(yout[:, 0:2, :], y[:])
            else:
                nc.vector.tensor_copy(yout[:, 2 * qp : 2 * qp + 2, :], y[:])
            if last:
                # split the final store so the first half flies while the
                # second egress copy is still running
                nc.gpsimd.dma_start(
                    out[b].rearrange("(g p) e -> p g e", p=P)[
                        :, 2 * qp : 2 * qp + 2, :
                    ],
                    yout[:, 2 * qp : 2 * qp + 2, :],
                )
        if not last:
            nc.gpsimd.dma_start(
                out[b].rearrange("(g p) e -> p g e", p=P), yout[:]
            )
        del batch_states[b]

    # Software pipeline, 3 stages deep: heads (QK) run 2 units ahead of the
    # exp/PV front-tails, and each head's normalize trails its PV by one more
    # unit. The PE queue always has two heads of QK ready, and every
    # normalize op's dependencies are settled before it is dequeued.
    units = [(b, h, rep) for rep in range(repeat) for b in range(BPC) for h in range(H)]
    proj_seq["total"] = BPC * repeat
    heads = []  # states awaiting their front-tail (2 units behind)
    norms = []  # states awaiting their normalize (1 further unit behind)
    projs = []  # batches awaiting their projection
    for unit in units:
        heads.append((emit_head(unit[:2], unit[2]), unit[2]))
        if len(heads) > 2:
            state, trep = heads.pop(0)
            norms.append((emit_tail(state, trep), trep))
        if len(norms) > 1:
            nstate, trep = norms.pop(0)
            pstate = emit_norm(nstate, trep)
            if pstate is not None:
                projs.append((pstate, trep))
            elif projs:
                emit_proj(*projs.pop(0))
    while heads or norms or projs:
        if heads:
            state, trep = heads.pop(0)
            norms.append((emit_tail(state, trep), trep))
        if norms:
            nstate, trep = norms.pop(0)
            pstate = emit_norm(nstate, trep, borrow_st=True)
            if pstate is not None:
                projs.append((pstate, trep))
                continue
        if projs:
            emit_proj(*projs.pop(0))


def build_nc(bpc=BPC, repeat=1):
    from contextlib import ExitStack

    nc = bacc.Bacc("TRN2", target_bir_lowering=False, debug=False)
    qT = nc.dram_tensor("qT", [bpc, E, S], TD, kind="ExternalInput").ap()
    kT = nc.dram_tensor("kT", [bpc, E, S], TD, kind="ExternalInput").ap()
    v = nc.dram_tensor("v", [bpc, S, E], TD, kind="ExternalInput").ap()
    wT = nc.dram_tensor("wT", [E, E], TD, kind="ExternalInput").ap()
    bo = nc.dram_tensor("bo", [1, E], FP, kind="ExternalInput").ap()
    out = nc.dram_tensor("out", [bpc, S, E], BF, kind="ExternalOutput").ap()

    with tile.TileContext(nc) as tc:
        with ExitStack() as ctx:
            saved = globals()["BPC"]
            globals()["BPC"] = bpc
            try:
                attention_kernel(ctx, tc, out, qT, kT, v, wT, bo, repeat=repeat)
            finally:
                globals()["BPC"] = saved
    nc.compile()
    return nc


def _np_td():
    import ml_dtypes

    return np.dtype(ml_dtypes.bfloat16)


def make_in_maps(qry, key, val, w_out, b_out):
    td = _np_td()
    qT_all = np.ascontiguousarray(qry.transpose(0, 2, 1)).astype(td)
    kT_all = np.ascontiguousarray(key.transpose(0, 2, 1)).astype(td)
    val = val.astype(td)
    wT = np.ascontiguousarray(w_out.T).astype(td)
    bo = np.ascontiguousarray(b_out.reshape(1, E), dtype=np.float32)
    maps = []
    for c in range(NCORES):
        sl = slice(c * BPC, (c + 1) * BPC)
        maps.append(
            {
                "qT": qT_all[sl],
                "kT": kT_all[sl],
                "v": np.ascontiguousarray(val[sl]),
                "wT": wT,
                "bo": bo,
            }
        )
    return maps


_NC_CACHE = {}


def _get_nc():
    if "nc" not in _NC_CACHE:
        _NC_CACHE["nc"] = build_nc()
    return _NC_CACHE["nc"]


def kernel(qry, key, val, w_out, b_out, **run_kwargs):
    nc = _get_nc()
    in_maps = make_in_maps(
        np.asarray(qry, dtype=np.float32),
        np.asarray(key, dtype=np.float32),
        np.asarray(val, dtype=np.float32),
        np.asarray(w_out, dtype=np.float32),
        np.asarray(b_out, dtype=np.float32),
    )
    res = run_bass_kernel_spmd(nc, in_maps, core_ids=list(range(NCORES)), **run_kwargs)
    out = np.concatenate(
        [res.results[c]["out"].astype(np.float32) for c in range(NCORES)], axis=0
    )
    out += np.asarray(b_out, dtype=np.float32)
    if run_kwargs:
        kernel.last_results = res
    return out



# revision 10
# speedup vs baseline: 1.0113x; 1.0038x over previous
"""Causal MHA + out-projection Trainium2 kernel (B=64, S=512, E=256, H=4).

Sharding: data-parallel over batch, 8 batches per NeuronCore x 8 cores.

Per (batch, head):
  - S^T[k, q] = sum_d K[k,d] Q[q,d]  (chunks of 128 keys; q free dim,
    causally restricted to q >= 128*j for key-chunk j)
  - P^T = exp(S^T/8) on the scalar engine (one op per head); the causal
    triangle inside each 128x128 diagonal block is zeroed afterwards with
    one gpsimd affine_select per chunk (cheaper than mask matmuls on PE).
  - PV in q-partition layout: per q-tile i, key-chunks j<=i accumulate
    matmul(lhsT = q-tile slice of P^T, rhs = [V_h | one ones-column]) into
    O[128q, 4, 65] -- a single PSUM bank, with the softmax denominator
    landing as one per-partition column per q-tile.
  - Normalize is therefore tiny on the vector engine: a 4-column
    reciprocal_approx_fast reading the denominator column straight from
    PSUM, and ONE broadcast multiply (stride-0 free dim) writing this
    head's 64-column half of a PAIR tile X[q, 4 tiles, 128] in bf16.
  - Four [128,128] PE transpose-mode matmuls per head PAIR (not per head:
    a transpose costs its 128 output columns regardless of partition
    count, so stacking both heads' d-halves on the partition dim halves
    the transpose count) restore X^T[d-pair, q] into bf16 PSUM staging
    tag-shared with the projection output, evacuated once per pair.
  - Y[q,:] = X^T.T @ W^T in one reused 1-bank PSUM slot, egressed as bf16 on
    the vector engine; the bias row is added on the host after the gather
    (a per-feature constant -- exact up to the bf16 output rounding).

The emission is software-pipelined 3 stages deep (QK heads 2 units ahead of
the exp/PV tails, normalize one further unit behind) so every op's
dependencies are settled before it reaches the front of its strict-FIFO
engine queue. Host-side prep: qry/key are pre-transposed to [B, E, S] and
w_out to W^T so that every device DMA is contiguous.
"""

import sys

if "/opt/trn_rl_repo" not in sys.path:
    sys.path.insert(0, "/opt/trn_rl_repo")

import numpy as np

import concourse.bass as bass
import concourse.mybir as mybir
import concourse.tile as tile
from concourse import bacc
from concourse.bass_utils import run_bass_kernel_spmd

B, S, E, H = 64, 512, 256, 4
D = E // H  # 64
P = 128
NCORES = 8
BPC = B // NCORES  # 8

FP = mybir.dt.float32
BF = mybir.dt.bfloat16
TD = BF

# S^T chunk -> column offset inside the 3-bank (1280 col) psum tile.
# chunk j covers keys [128j, 128j+128), q in [128j, 512) => width 512-128j.
CHUNK_COL = [0, 512, 1024, 896]
ST_COLS = 1280  # exp() span


def attention_kernel(ctx, tc, out, qT, kT, v, wT, bo, repeat=1):
    nc = tc.nc
    AF = mybir.ActivationFunctionType
    OP = mybir.AluOpType

    consts = ctx.enter_context(tc.tile_pool(name="consts", bufs=1))
    qk_pool = ctx.enter_context(tc.tile_pool(name="qk", bufs=3))
    v_pool = ctx.enter_context(tc.tile_pool(name="v", bufs=3))
    pt_pool = ctx.enter_context(tc.tile_pool(name="pt", bufs=3))
    xt_pool = ctx.enter_context(tc.tile_pool(name="xt", bufs=2))
    yo_pool = ctx.enter_context(tc.tile_pool(name="yo", bufs=2))
    r_pool = ctx.enter_context(tc.tile_pool(name="rsb", bufs=3))
    st_psum = ctx.enter_context(tc.tile_pool(name="st", bufs=2, space="PSUM"))
    # q-layout PV output: [128 q, 4 tiles, 65] fp32 = 1040B -> one bank.
    o_psum = ctx.enter_context(tc.tile_pool(name="opv", bufs=1, space="PSUM"))
    # X^T transpose staging ([128,512] fp32) and proj output y ([128,2,256]
    # fp32) are both 2KB/partition and share the last psum bank.
    oy_psum = ctx.enter_context(tc.tile_pool(name="oy", bufs=1, space="PSUM"))

    # W^T DMA is emitted lazily (first emit_proj) so the first batch's q/k
    # loads go ahead of it on the sync DMA queue and PE starts sooner.
    wt_state = {}

    def get_wt():
        if "wt" not in wt_state:
            wt_sb = consts.tile([P, 2, E], TD)
            nc.sync.dma_start(wt_sb[:], wT.rearrange("(c p) e -> p c e", p=P))
            wt_state["wt"] = wt_sb
        return wt_state["wt"]

    zero_reg = nc.gpsimd.to_reg(0.0)

    # 128x128 bf16 identity for PE transpose-mode
    ident = consts.tile([P, P], BF, tag="ident")
    nc.gpsimd.memset(ident[:], 1.0)
    nc.gpsimd.affine_select(
        out=ident[:], in_=ident[:], compare_op=OP.is_equal, fill=zero_reg,
        base=0, pattern=[[1, P]], channel_multiplier=-1,
    )

    # Warm-up: a 1-column matmul with no DMA dependency absorbs the
    # first-matmul pipeline/dispatch cost while the q/k loads are in flight.
    warm = st_psum.tile([P, ST_COLS], FP, tag="st", name="warmup")
    nc.tensor.matmul(
        warm[0:1, 0:1], lhsT=ident[:, 0:1], rhs=ident[:, 0:1], start=True,
        stop=True,
    )

    # ---- software-pipelined emission over (batch, head) units ----
    # Emitting S^T(h+1) before the exp/PV/normalize tail of h keeps the
    # in-order PE queue free of head-of-line stalls (PE always has the next
    # head's QK matmuls ready while ACT/DVE drain the previous head).
    batch_states = {}

    def emit_head(bu, rep):
        b, h = bu
        if h == 0:
            # q/k loads split by E-half: heads 0/1 only need the c=0 half,
            # so their QK starts after half the load bytes
            qt_sb = qk_pool.tile([P, 2, S], TD, tag="qt", name=f"qt{b}_{rep}")
            kt_sb = qk_pool.tile([P, 2, S], TD, tag="kt", name=f"kt{b}_{rep}")
            qT_r = qT[b].rearrange("(c p) s -> p c s", p=P)
            kT_r = kT[b].rearrange("(c p) s -> p c s", p=P)
            for c in range(2):
                nc.sync.dma_start(qt_sb[:, c, :], qT_r[:, c, :])
                nc.sync.dma_start(kt_sb[:, c, :], kT_r[:, c, :])
            # V loads ride the sync queue (Pool carries the affine_select
            # masks and is the busier of the two); stores stay on Pool
            va = v_pool.tile([P, 4, H, P], TD, tag="va", name=f"va{b}_{rep}")
            nc.gpsimd.memset(va[:, :, :, D:], 1.0)
            for j in range(4):
                nc.sync.dma_start(
                    va[:, j, :, :D],
                    v[b, P * j : P * (j + 1)].rearrange("p (h d) -> p h d", d=D),
                )
            xt_sb = xt_pool.tile([P, 2, S], TD, tag="xt", name=f"xt{b}_{rep}")
            batch_states[b] = (qt_sb, kt_sb, va, xt_sb)
        qt_sb, kt_sb, va, xt_sb = batch_states[b]

        hp = D * (h % 2)
        hc = h // 2
        st = st_psum.tile([P, ST_COLS], FP, tag="st", name=f"st{b}_{h}_{rep}")
        for j in range(4):
            qoff = P * j
            qr = S - qoff
            col = CHUNK_COL[j]
            nc.tensor.matmul(
                st[:, col : col + qr],
                lhsT=kt_sb[hp : hp + D, hc, qoff : qoff + P],
                rhs=qt_sb[hp : hp + D, hc, qoff:S],
                start=True,
                stop=True,
            )
        return (b, h, st, va, xt_sb)

    def emit_tail(state, rep):
        b, h, st, va, xt_sb = state
        hp = D * (h % 2)
        hc = h // 2
        pt = pt_pool.tile([P, ST_COLS], TD, tag="pt", name=f"pt{b}_{h}_{rep}")
        nc.scalar.activation(pt[:], st[:, 0:ST_COLS], AF.Exp, scale=0.125)
        # zero the causal triangle of each 128x128 diagonal block:
        # keep iff q_rel - k >= 0 (k = partition, q_rel = col in block)
        for j in range(4):
            col = CHUNK_COL[j]
            nc.gpsimd.affine_select(
                out=pt[:, col : col + P],
                in_=pt[:, col : col + P],
                compare_op=OP.is_ge,
                fill=zero_reg,
                base=0,
                pattern=[[1, P]],
                channel_multiplier=-1,
            )

        # PV in q-partition layout: per q-tile i, accumulate key-chunks
        # j<=i with lhsT = the q-tile slice of P^T and rhs = [V_h | one
        # ones-column] -> O[q, 0:64] plus the softmax denominator in col 64.
        opv = o_psum.tile([P, 4, 65], FP, tag="opv", name=f"o{b}_{h}_{rep}")
        for i in range(4):
            for j in range(i + 1):
                col = CHUNK_COL[j] + P * (i - j)
                nc.tensor.matmul(
                    opv[:, i, :],
                    lhsT=pt[:, col : col + P],
                    rhs=va[:, j, h, 0:65],
                    start=(j == 0),
                    stop=(j == i),
                )

        return (b, h, opv, xt_sb)

    pair_xt = {}

    def emit_norm(nstate, rep, borrow_st=False):
        b, h, opv, xt_sb = nstate
        hp = D * (h % 2)
        hc = h // 2
        # normalize in q-layout: the denominator is one column per q-tile, so
        # the reciprocal runs on 4 elements/partition instead of 512, and the
        # divide is one broadcast multiply. Then 4 PE transposes restore the
        # X^T layout the projection needs (psum staged through the shared
        # oy bank, evacuated once per head pair).
        r4 = r_pool.tile([P, 4], FP, tag="rsb", name=f"r{b}_{h}_{rep}")
        nc.vector.reciprocal_approx_fast(r4[:], opv[:, :, 64])
        # normalize into the PAIR tile: head h fills columns [hp, hp+64) of
        # each q-tile block. At h odd, FOUR [128,128] transposes handle the
        # whole pair (the transpose's cost is its 128 output columns either
        # way -- stacking both heads' d-halves on the partition dim halves
        # the transpose count, cutting ~3.4us off the PE wall).
        if h % 2 == 0:
            pair_xt["x"] = r_pool.tile(
                [P, 4, P], BF, tag="xsb", name=f"x{b}_{hc}_{rep}"
            )
        x_pair = pair_xt["x"]
        nc.vector.tensor_tensor(
            x_pair[:, :, hp : hp + D],
            opv[:, :, 0:D],
            r4[:].unsqueeze(2).broadcast_to((P, 4, D)),
            OP.mult,
        )
        if h % 2 == 1:
            if borrow_st:
                # drain time: the st banks are idle, so the last pair's
                # transpose staging borrows one instead of waiting for the
                # previous pair's evacuation of the shared oy bank
                xst = st_psum.tile([P, ST_COLS], FP, tag="st", name=f"xb{b}_{rep}")
                xt_ps = xst[:, 0:512].bitcast(BF).rearrange(
                    "p (two i c) -> p two i c", two=2, c=P
                )
            else:
                # bf16 transpose staging padded to 2KB so it tag-shares the
                # single oy psum bank with the fp32 projection output
                xt_ps = oy_psum.tile(
                    [P, 2, 4, P], BF, tag="oy", name=f"xp{b}_{hc}_{rep}"
                )
            for i in range(4):
                nc.tensor.transpose(
                    xt_ps[:, 0, i, :], x_pair[:, i, :], ident[:]
                )
            if borrow_st:
                # drain: ACT and DVE are both winding down; split the
                # evacuation across them so the final projections (gated on
                # this copy) start half an op earlier
                nc.scalar.copy(xt_sb[:, hc, 0:256], xt_ps[:, 0, 0:2, :])
                nc.vector.tensor_copy(xt_sb[:, hc, 256:512], xt_ps[:, 0, 2:4, :])
            else:
                nc.vector.tensor_copy(xt_sb[:, hc, :], xt_ps[:, 0, :, :])

        return (b, xt_sb) if h == H - 1 else None

    proj_seq = {"n": 0, "total": None}

    def emit_proj(pstate, rep):
        b, xt_sb = pstate
        proj_seq["n"] += 1
        last = proj_seq["n"] == proj_seq["total"]
        # out-projection, two 1-bank psum halves; egress split across the
        # scalar and vector engines (bias is added host-side)
        yout = yo_pool.tile([P, 4, E], BF, tag="yout", name=f"yo{b}_{rep}")
        # ONE psum slot serves both projection halves (qp1 overwrites after
        # qp0's egress) so the second oy slot stays free and the next batch's
        # PV never waits on this batch's y egress -- that round-trip was a
        # 1.8us DVE stall at every batch boundary. The LAST projection has
        # nothing behind it, so it takes both slots and splits its egress
        # across the scalar and vector engines to shorten the drain.
        y = oy_psum.tile([P, 2, E], FP, tag="oy", name=f"y{b}_{rep}")
        for qp in range(2):
            if last and qp == 1:
                # the st banks are idle during the drain: borrow one so the
                # final qp1 projection never waits on qp0's egress copy
                y_st = st_psum.tile([P, ST_COLS], FP, tag="st", name=f"y{b}b_{rep}")
                y = y_st[:, 0 : 2 * E].rearrange("p (c e) -> p c e", c=2)
            for qi in range(2):
                qt_idx = 2 * qp + qi
                for c in range(2):
                    nc.tensor.matmul(
                        y[:, qi, :],
                        lhsT=xt_sb[:, c, P * qt_idx : P * (qt_idx + 1)],
                        rhs=get_wt()[:, c, :],
                        start=(c == 0),
                        stop=(c == 1),
                    )
            if last and qp == 0:
                nc.scalar.copy(yout[:, 0:2, :], y[:])
            else:
                nc.vector.tensor_copy(yout[:, 2 * qp : 2 * qp + 2, :], y[:])
            if last:
                # split the final store so the first half flies while the
                # second egress copy is still running; the TERMINAL half
                # issues from the (idle) sync queue so it never waits behind
                # the first half's issue on Pool
                eng = nc.gpsimd if qp == 0 else nc.sync
                eng.dma_start(
                    out[b].rearrange("(g p) e -> p g e", p=P)[
                        :, 2 * qp : 2 * qp + 2, :
                    ],
                    yout[:, 2 * qp : 2 * qp + 2, :],
                )
        if not last:
            nc.gpsimd.dma_start(
                out[b].rearrange("(g p) e -> p g e", p=P), yout[:]
            )
        del batch_states[b]

    # Software pipeline, 3 stages deep: heads (QK) run 2 units ahead of the
    # exp/PV front-tails, and each head's normalize trails its PV by one more
    # unit. The PE queue always has two heads of QK ready, and every
    # normalize op's dependencies are settled before it is dequeued.
    units = [(b, h, rep) for rep in range(repeat) for b in range(BPC) for h in range(H)]
    proj_seq["total"] = BPC * repeat
    heads = []  # states awaiting their front-tail (2 units behind)
    norms = []  # states awaiting their normalize (1 further unit behind)
    projs = []  # batches awaiting their projection
    for unit in units:
        heads.append((emit_head(unit[:2], unit[2]), unit[2]))
        if len(heads) > 2:
            state, trep = heads.pop(0)
            norms.append((emit_tail(state, trep), trep))
        if len(norms) > 1:
            nstate, trep = norms.pop(0)
            pstate = emit_norm(nstate, trep)
            if pstate is not None:
                projs.append((pstate, trep))
            elif projs:
                emit_proj(*projs.pop(0))
    while heads or norms or projs:
        if heads:
            state, trep = heads.pop(0)
            norms.append((emit_tail(state, trep), trep))
        if norms:
            nstate, trep = norms.pop(0)
            pstate = emit_norm(nstate, trep, borrow_st=True)
            if pstate is not None:
                projs.append((pstate, trep))
                continue
        if projs:
            emit_proj(*projs.pop(0))


def build_nc(bpc=BPC, repeat=1):
    from contextlib import ExitStack

    nc = bacc.Bacc("TRN2", target_bir_lowering=False, debug=False)
    qT = nc.dram_tensor("qT", [bpc, E, S], TD, kind="ExternalInput").ap()
    kT = nc.dram_tensor("kT", [bpc, E, S], TD, kind="ExternalInput").ap()
    v = nc.dram_tensor("v", [bpc, S, E], TD, kind="ExternalInput").ap()
    wT = nc.dram_tensor("wT", [E, E], TD, kind="ExternalInput").ap()
    bo = nc.dram_tensor("bo", [1, E], FP, kind="ExternalInput").ap()
    out = nc.dram_tensor("out", [bpc, S, E], BF, kind="ExternalOutput").ap()

    with tile.TileContext(nc) as tc:
        with ExitStack() as ctx:
            saved = globals()["BPC"]
            globals()["BPC"] = bpc
            try:
                attention_kernel(ctx, tc, out, qT, kT, v, wT, bo, repeat=repeat)
            finally:
                globals()["BPC"] = saved
    nc.compile()
    return nc


def _np_td():
    import ml_dtypes

    return np.dtype(ml_dtypes.bfloat16)


def make_in_maps(qry, key, val, w_out, b_out):
    td = _np_td()
    qT_all = np.ascontiguousarray(qry.transpose(0, 2, 1)).astype(td)
    kT_all = np.ascontiguousarray(key.transpose(0, 2, 1)).astype(td)
    val = val.astype(td)
    wT = np.ascontiguousarray(w_out.T).astype(td)
    bo = np.ascontiguousarray(b_out.reshape(1, E), dtype=np.float32)
    maps = []
    for c in range(NCORES):
        sl = slice(c * BPC, (c + 1) * BPC)
        maps.append(
            {
                "qT": qT_all[sl],
                "kT": kT_all[sl],
                "v": np.ascontiguousarray(val[sl]),
                "wT": wT,
                "bo": bo,
            }
        )
    return maps


_NC_CACHE = {}


def _get_nc():
    if "nc" not in _NC_CACHE:
        _NC_CACHE["nc"] = build_nc()
    return _NC_CACHE["nc"]


def kernel(qry, key, val, w_out, b_out, **run_kwargs):
    nc = _get_nc()
    in_maps = make_in_maps(
        np.asarray(qry, dtype=np.float32),
        np.asarray(key, dtype=np.float32),
        np.asarray(val, dtype=np.float32),
        np.asarray(w_out, dtype=np.float32),
        np.asarray(b_out, dtype=np.float32),
    )
    res = run_bass_kernel_spmd(nc, in_maps, core_ids=list(range(NCORES)), **run_kwargs)
    out = np.concatenate(
        [res.results[c]["out"].astype(np.float32) for c in range(NCORES)], axis=0
    )
    out += np.asarray(b_out, dtype=np.float32)
    if run_kwargs:
        kernel.last_results = res
    return out

